# revision 2
# baseline (speedup 1.0000x reference)
"""MoE SwiGLU experts kernel for Trainium2, 8 NeuronCores.

Strategy: expert-pair parallel with F-split.
  - Tokens are sorted by expert on the host (argsort of expert_idx).
  - Cores 2i and 2i+1 jointly own experts (2i, 2i+1): both cores get the
    same token set (experts 2i & 2i+1, padded to `cap` tokens each), but
    core 2i uses columns [0:2048] of w_gate/w_up (rows of w_down) and
    core 2i+1 uses columns [2048:4096].  Each core runs the full SwiGLU
    over its F-half; the host sums the two partial down-projections.
  - On-chip layout is feature-major (partitions = feature dim, free dim =
    tokens), so w_gate/w_up/w_down natural layouts serve directly as the
    stationary lhsT operand: out = lhsT.T @ rhs.  Zero on-chip transposes.
"""

import math

import numpy as np
import orjson

import concourse.bass as bass
import concourse.mybir as mybir
import concourse.tile as tile
from concourse import bass2jax
from concourse.bass_utils import run_bass_kernel_spmd

FP32 = mybir.dt.float32

D = 1024        # model dim
F = 4096        # ffn dim
F2 = F // 2     # per-core F half
E = 8           # experts
KD = D // 128   # 8  k-tiles over D
KF = F2 // 128  # 16 f-tiles over F half


# ---------------------------------------------------------------------------
# BIR legalizer: this container's walrus accepts at most ONE sync-wait per
# instruction.  Tile emits instructions with several waits; hoist the excess
# onto preceding EventSemaphore carrier instructions on the same engine
# (engines execute their stream in order, so waiting earlier is equivalent).
# ---------------------------------------------------------------------------

def _legalize_bir_waits(bir_bytes: bytes) -> bytes:
    bir = orjson.loads(bir_bytes)
    n_fix = 0
    for f in bir.get("functions", []):
        for b in f.get("blocks", []):
            out = []
            for inst in b.get("instructions", []):
                si = inst.get("sync_info")
                waits = (si or {}).get("on_wait") or []
                if len(waits) > 1:
                    keep = waits[-1:]
                    excess = waits[:-1]
                    for w in excess:
                        n_fix += 1
                        out.append({
                            "debug": inst.get("debug", 0),
                            "engine": inst["engine"],
                            "ins": [], "outs": [],
                            "name": f"wfix-{n_fix}-{inst['name']}",
                            "opcode": "EventSemaphore",
                            "sync_info": {"on_update": [], "on_wait": [w]},
                        })
                    si["on_wait"] = keep
                out.append(inst)
            b["instructions"] = out
    return orjson.dumps(bir)


_orig_decompress = bass2jax._decompress_ant_bir


def _patched_decompress(v):
    return _legalize_bir_waits(_orig_decompress(v))


bass2jax._decompress_ant_bir = _patched_decompress


# ---------------------------------------------------------------------------
# Device program (SPMD, identical across the 8 cores; per-core inputs differ)
# ---------------------------------------------------------------------------

def _chunks(cap):
    """Split the token free-dim into matmul-legal chunks (<=512 fp32)."""
    out = []
    c0 = 0
    while c0 < cap:
        w = min(512, cap - c0)
        out.append((c0, w))
        c0 += w
    return out


def build_program(cap: int) -> bass.Bass:
    nc = bass.Bass()

    # unit a = even expert of the pair, unit b = odd expert.
    xa = nc.declare_dram_parameter("xa", [KD, 128, cap], FP32, isOutput=False)
    xb = nc.declare_dram_parameter("xb", [KD, 128, cap], FP32, isOutput=False)
    wga = nc.declare_dram_parameter("wga", [KF, 128, KD, 128], FP32, isOutput=False)
    wua = nc.declare_dram_parameter("wua", [KF, 128, KD, 128], FP32, isOutput=False)
    wda = nc.declare_dram_parameter("wda", [KD, 128, KF, 128], FP32, isOutput=False)
    wgb = nc.declare_dram_parameter("wgb", [KF, 128, KD, 128], FP32, isOutput=False)
    wub = nc.declare_dram_parameter("wub", [KF, 128, KD, 128], FP32, isOutput=False)
    wdb = nc.declare_dram_parameter("wdb", [KD, 128, KF, 128], FP32, isOutput=False)
    ya = nc.declare_dram_parameter("ya", [KD, 128, cap], FP32, isOutput=True)
    yb = nc.declare_dram_parameter("yb", [KD, 128, cap], FP32, isOutput=True)

    units = [(xa, wga, wua, wda, ya), (xb, wgb, wub, wdb, yb)]
    chunks = _chunks(cap)

    with tile.TileContext(nc) as tc:
        with (
            tc.tile_pool(name="xp", bufs=1) as xp,
            tc.tile_pool(name="hp", bufs=1) as hp,
            tc.tile_pool(name="wp", bufs=3) as wp,
            tc.tile_pool(name="sp", bufs=2) as sp,
            tc.tile_pool(name="yp", bufs=2) as yp,
            tc.tile_pool(name="ps", bufs=2, space=bass.MemorySpace.PSUM) as ps,
        ):
            for x_d, wg_d, wu_d, wd_d, y_d in units:
                # token activations, feature-major: x_t[k] = X^T[d-tile k]
                x_t = []
                for k in range(KD):
                    xt = xp.tile([128, cap], FP32, tag=f"x{k}")
                    nc.sync.dma_start(xt[:], x_d[k])
                    x_t.append(xt)

                # H^T for this unit: [f (128 part), f-tile (16), tokens]
                h = hp.tile([128, KF, cap], FP32, tag="h")

                # ---- phase 1: G/U = X @ Wg, X @ Wu ; H = silu(G) * U ----
                for fj in range(KF):
                    wgt = wp.tile([128, KD, 128], FP32, tag="wg")
                    nc.sync.dma_start(wgt[:], wg_d[fj])
                    wut = wp.tile([128, KD, 128], FP32, tag="wu")
                    nc.sync.dma_start(wut[:], wu_d[fj])
                    for (c0, w) in chunks:
                        g_ps = ps.tile([128, w], FP32, tag="g")
                        for k in range(KD):
                            nc.tensor.matmul(
                                g_ps[:, :w], wgt[:, k, :], x_t[k][:, c0:c0 + w],
                                start=(k == 0), stop=(k == KD - 1),
                            )
                        u_ps = ps.tile([128, w], FP32, tag="u")
                        for k in range(KD):
                            nc.tensor.matmul(
                                u_ps[:, :w], wut[:, k, :], x_t[k][:, c0:c0 + w],
                                start=(k == 0), stop=(k == KD - 1),
                            )
                        sg = sp.tile([128, w], FP32, tag="sg")
                        nc.scalar.activation(
                            sg[:, :w], g_ps[:, :w],
                            mybir.ActivationFunctionType.Silu,
                        )
                        nc.vector.tensor_mul(
                            h[:, fj, c0:c0 + w], sg[:, :w], u_ps[:, :w]
                        )

                # ---- phase 2: Y^T partial = Wd^T @ H^T (accum over f) ----
                for d in range(KD):
                    wdt = wp.tile([128, KF, 128], FP32, tag="wd")
                    nc.sync.dma_start(wdt[:], wd_d[d])
                    y_sb = yp.tile([128, cap], FP32, tag="y")
                    for (c0, w) in chunks:
                        y_ps = ps.tile([128, w], FP32, tag="yp")
                        for fj in range(KF):
                            nc.tensor.matmul(
                                y_ps[:, :w], wdt[:, fj, :], h[:, fj, c0:c0 + w],
                                start=(fj == 0), stop=(fj == KF - 1),
                            )
                        nc.vector.tensor_copy(y_sb[:, c0:c0 + w], y_ps[:, :w])
                    nc.sync.dma_start(y_d[d], y_sb[:])

    return nc


# ---------------------------------------------------------------------------
# Host-side sharding / unsharding
# ---------------------------------------------------------------------------

def _prep_w_gate_like(w_slice):
    """[D, F2] -> [KF, 128, KD, 128] so each f-tile is one contiguous DMA."""
    arr = w_slice.reshape(KD, 128, KF, 128)       # [k, p, fj, f]
    return np.ascontiguousarray(arr.transpose(2, 1, 0, 3))


def _prep_w_down(w_slice):
    """[F2, D] -> [KD, 128, KF, 128] so each d-tile is one contiguous DMA."""
    arr = w_slice.reshape(KF, 128, KD, 128)       # [fj, p, d, dc]
    return np.ascontiguousarray(arr.transpose(2, 1, 0, 3))


def _prep_x(x_tok, cap):
    """[count, D] tokens -> [KD, 128, cap] feature-major padded."""
    count = x_tok.shape[0]
    xt = np.zeros((D, cap), dtype=np.float32)
    xt[:, :count] = x_tok.T
    return np.ascontiguousarray(xt.reshape(KD, 128, cap))


_prog_cache = {}


def get_program(cap: int) -> bass.Bass:
    if cap not in _prog_cache:
        _prog_cache[cap] = build_program(cap)
    return _prog_cache[cap]


def prepare_in_maps(np_inputs):
    """Host-side sharding.  Returns (in_maps, cap, meta) where meta carries
    what's needed to unshard."""
    x = np.asarray(np_inputs["x"])
    B, S, _ = x.shape
    xf = np.ascontiguousarray(x.reshape(-1, D).astype(np.float32, copy=False))
    idx = np.asarray(np_inputs["expert_idx"]).reshape(-1)
    w_gate = np.asarray(np_inputs["w_gate"], dtype=np.float32)
    w_up = np.asarray(np_inputs["w_up"], dtype=np.float32)
    w_down = np.asarray(np_inputs["w_down"], dtype=np.float32)

    order = np.argsort(idx, kind="stable")
    counts = np.bincount(idx, minlength=E).astype(np.int64)
    starts = np.zeros(E + 1, dtype=np.int64)
    np.cumsum(counts, out=starts[1:])
    cap = max(128, int(math.ceil(max(counts.max(), 1) / 128)) * 128)

    tok_of = [order[starts[e]:starts[e + 1]] for e in range(E)]
    x_of = [_prep_x(xf[tok_of[e]], cap) for e in range(E)]

    in_maps = []
    for pair in range(4):
        ea, eb = 2 * pair, 2 * pair + 1
        for hhalf in range(2):
            sl = slice(hhalf * F2, (hhalf + 1) * F2)
            in_maps.append({
                "xa": x_of[ea],
                "xb": x_of[eb],
                "wga": _prep_w_gate_like(w_gate[ea][:, sl]),
                "wua": _prep_w_gate_like(w_up[ea][:, sl]),
                "wda": _prep_w_down(w_down[ea][sl, :]),
                "wgb": _prep_w_gate_like(w_gate[eb][:, sl]),
                "wub": _prep_w_gate_like(w_up[eb][:, sl]),
                "wdb": _prep_w_down(w_down[eb][sl, :]),
            })
    meta = (tok_of, counts, (B, S), x.dtype)
    return in_maps, cap, meta


def unshard(results, cap, meta):
    tok_of, counts, (B, S), out_dtype = meta
    out = np.zeros((B * S, D), dtype=np.float32)
    for pair in range(4):
        ea, eb = 2 * pair, 2 * pair + 1
        r0 = results[2 * pair]
        r1 = results[2 * pair + 1]
        for e, key in ((ea, "ya"), (eb, "yb")):
            yt = (np.asarray(r0[key]) + np.asarray(r1[key])).reshape(D, cap)
            out[tok_of[e]] = yt[:, :counts[e]].T
    return out.reshape(B, S, D).astype(out_dtype, copy=False)


def kernel(x, expert_idx, w_gate, w_up, w_down):
    np_inputs = {"x": x, "expert_idx": expert_idx, "w_gate": w_gate,
                 "w_up": w_up, "w_down": w_down}
    in_maps, cap, meta = prepare_in_maps(np_inputs)
    nc = get_program(cap)
    res = run_bass_kernel_spmd(nc, in_maps, list(range(8)))
    return unshard(res.results, cap, meta)


# revision 5
# speedup vs baseline: 2.9295x; 2.9295x over previous
"""MoE SwiGLU experts kernel for Trainium2, 8 NeuronCores.

Strategy: expert-pair parallel with F-split.
  - Tokens are sorted by expert on the host (argsort of expert_idx).
  - Cores 2i and 2i+1 jointly own experts (2i, 2i+1): both cores get the
    same token set (experts 2i & 2i+1, padded to `cap` tokens each), but
    core 2i uses columns [0:2048] of w_gate/w_up (rows of w_down) and
    core 2i+1 uses columns [2048:4096].  Each core runs the full SwiGLU
    over its F-half; the host sums the two partial down-projections.
  - On-chip layout is feature-major (partitions = feature dim, free dim =
    tokens), so w_gate/w_up/w_down natural layouts serve directly as the
    stationary lhsT operand: out = lhsT.T @ rhs.  Zero on-chip transposes.
"""

import math

import numpy as np
import orjson

import concourse.bass as bass
import concourse.mybir as mybir
import concourse.tile as tile
from concourse import bass2jax
from concourse.bass_utils import run_bass_kernel_spmd

FP32 = mybir.dt.float32
FP32R = mybir.dt.float32r

D = 1024        # model dim
F = 4096        # ffn dim
F2 = F // 2     # per-core F half
E = 8           # experts
KD = D // 128   # 8  k-tiles over D
KF = F2 // 128  # 16 f-tiles over F half


# ---------------------------------------------------------------------------
# BIR legalizer: this container's walrus accepts at most ONE sync-wait per
# instruction.  Tile emits instructions with several waits; hoist the excess
# onto preceding EventSemaphore carrier instructions on the same engine
# (engines execute their stream in order, so waiting earlier is equivalent).
# ---------------------------------------------------------------------------

def _legalize_bir_waits(bir_bytes: bytes) -> bytes:
    bir = orjson.loads(bir_bytes)
    n_fix = 0
    for f in bir.get("functions", []):
        for b in f.get("blocks", []):
            out = []
            for inst in b.get("instructions", []):
                si = inst.get("sync_info")
                waits = (si or {}).get("on_wait") or []
                if len(waits) > 1:
                    keep = waits[-1:]
                    excess = waits[:-1]
                    for w in excess:
                        n_fix += 1
                        out.append({
                            "debug": inst.get("debug", 0),
                            "engine": inst["engine"],
                            "ins": [], "outs": [],
                            "name": f"wfix-{n_fix}-{inst['name']}",
                            "opcode": "EventSemaphore",
                            "sync_info": {"on_update": [], "on_wait": [w]},
                        })
                    si["on_wait"] = keep
                out.append(inst)
            b["instructions"] = out
    return orjson.dumps(bir)


_orig_decompress = bass2jax._decompress_ant_bir


def _patched_decompress(v):
    return _legalize_bir_waits(_orig_decompress(v))


bass2jax._decompress_ant_bir = _patched_decompress


# ---------------------------------------------------------------------------
# Device program (SPMD, identical across the 8 cores; per-core inputs differ)
# ---------------------------------------------------------------------------

def _chunks(cap):
    """Split the token free-dim into matmul chunks: <=512 wide (one fp32 PSUM
    bank), multiples of 128, as even as possible.  float32r matmuls run at
    full rate only when the moving free-dim is >=256, so even splits beat a
    512/512/128 tail split."""
    n = max(1, math.ceil(cap / 512))
    base = (cap // n) // 128 * 128
    rem = (cap - n * base) // 128
    widths = [base + 128] * rem + [base] * (n - rem)
    out = []
    c0 = 0
    for w in widths:
        out.append((c0, w))
        c0 += w
    return out


def build_program(cap: int) -> bass.Bass:
    nc = bass.Bass()

    # unit a = even expert of the pair, unit b = odd expert.
    xa = nc.declare_dram_parameter("xa", [KD, 128, cap], FP32R, isOutput=False)
    xb = nc.declare_dram_parameter("xb", [KD, 128, cap], FP32R, isOutput=False)
    wga = nc.declare_dram_parameter("wga", [KF, 128, KD, 128], FP32R, isOutput=False)
    wua = nc.declare_dram_parameter("wua", [KF, 128, KD, 128], FP32R, isOutput=False)
    wda = nc.declare_dram_parameter("wda", [KD, 128, KF, 128], FP32R, isOutput=False)
    wgb = nc.declare_dram_parameter("wgb", [KF, 128, KD, 128], FP32R, isOutput=False)
    wub = nc.declare_dram_parameter("wub", [KF, 128, KD, 128], FP32R, isOutput=False)
    wdb = nc.declare_dram_parameter("wdb", [KD, 128, KF, 128], FP32R, isOutput=False)
    ya = nc.declare_dram_parameter("ya", [KD, 128, cap], FP32, isOutput=True)
    yb = nc.declare_dram_parameter("yb", [KD, 128, cap], FP32, isOutput=True)

    units = [(xa, wga, wua, wda, ya), (xb, wgb, wub, wdb, yb)]
    chunks = _chunks(cap)

    with tile.TileContext(nc) as tc:
        with (
            tc.tile_pool(name="xp", bufs=1) as xp,
            tc.tile_pool(name="hp", bufs=1) as hp,
            tc.tile_pool(name="wp", bufs=3) as wp,
            tc.tile_pool(name="sp", bufs=2) as sp,
            tc.tile_pool(name="yp", bufs=2) as yp,
            tc.tile_pool(name="ps", bufs=2, space=bass.MemorySpace.PSUM) as ps,
        ):
            for x_d, wg_d, wu_d, wd_d, y_d in units:
                # token activations, feature-major: x_t[k] = X^T[d-tile k]
                x_t = []
                for k in range(KD):
                    xt = xp.tile([128, cap], FP32R, tag=f"x{k}")
                    nc.sync.dma_start(xt[:], x_d[k])
                    x_t.append(xt)

                # H^T for this unit: [f (128 part), f-tile (16), tokens]
                h = hp.tile([128, KF, cap], FP32R, tag="h")

                # ---- phase 1: G/U = X @ Wg, X @ Wu ; H = silu(G) * U ----
                for fj in range(KF):
                    wgt = wp.tile([128, KD, 128], FP32R, tag="wg")
                    nc.sync.dma_start(wgt[:], wg_d[fj])
                    wut = wp.tile([128, KD, 128], FP32R, tag="wu")
                    nc.sync.dma_start(wut[:], wu_d[fj])
                    for (c0, w) in chunks:
                        g_ps = ps.tile([128, w], FP32, tag="g")
                        for k in range(KD):
                            nc.tensor.matmul(
                                g_ps[:, :w],
                                wgt[:, k, :], x_t[k][:, c0:c0 + w],
                                start=(k == 0), stop=(k == KD - 1),
                            )
                        u_ps = ps.tile([128, w], FP32, tag="u")
                        for k in range(KD):
                            nc.tensor.matmul(
                                u_ps[:, :w],
                                wut[:, k, :], x_t[k][:, c0:c0 + w],
                                start=(k == 0), stop=(k == KD - 1),
                            )
                        sg = sp.tile([128, w], FP32, tag="sg")
                        nc.scalar.activation(
                            sg[:, :w], g_ps[:, :w],
                            mybir.ActivationFunctionType.Silu,
                        )
                        nc.vector.tensor_mul(
                            h[:, fj, c0:c0 + w], sg[:, :w], u_ps[:, :w]
                        )

                # ---- phase 2: Y^T partial = Wd^T @ H^T (accum over f) ----
                for d in range(KD):
                    wdt = wp.tile([128, KF, 128], FP32R, tag="wd")
                    nc.sync.dma_start(wdt[:], wd_d[d])
                    y_sb = yp.tile([128, cap], FP32, tag="y")
                    for (c0, w) in chunks:
                        y_ps = ps.tile([128, w], FP32, tag="yp")
                        for fj in range(KF):
                            nc.tensor.matmul(
                                y_ps[:, :w],
                                wdt[:, fj, :], h[:, fj, c0:c0 + w],
                                start=(fj == 0), stop=(fj == KF - 1),
                            )
                        nc.vector.tensor_copy(y_sb[:, c0:c0 + w], y_ps[:, :w])
                    nc.sync.dma_start(y_d[d], y_sb[:])

    return nc


# ---------------------------------------------------------------------------
# Host-side sharding / unsharding
# ---------------------------------------------------------------------------

def _prep_w_gate_like(w_slice):
    """[D, F2] -> [KF, 128, KD, 128] so each f-tile is one contiguous DMA."""
    arr = w_slice.reshape(KD, 128, KF, 128)       # [k, p, fj, f]
    return np.ascontiguousarray(arr.transpose(2, 1, 0, 3))


def _prep_w_down(w_slice):
    """[F2, D] -> [KD, 128, KF, 128] so each d-tile is one contiguous DMA."""
    arr = w_slice.reshape(KF, 128, KD, 128)       # [fj, p, d, dc]
    return np.ascontiguousarray(arr.transpose(2, 1, 0, 3))


def _prep_x(x_tok, cap):
    """[count, D] tokens -> [KD, 128, cap] feature-major padded."""
    count = x_tok.shape[0]
    xt = np.zeros((D, cap), dtype=np.float32)
    xt[:, :count] = x_tok.T
    return np.ascontiguousarray(xt.reshape(KD, 128, cap))


_prog_cache = {}


def get_program(cap: int) -> bass.Bass:
    if cap not in _prog_cache:
        _prog_cache[cap] = build_program(cap)
    return _prog_cache[cap]


def prepare_in_maps(np_inputs):
    """Host-side sharding.  Returns (in_maps, cap, meta) where meta carries
    what's needed to unshard."""
    x = np.asarray(np_inputs["x"])
    B, S, _ = x.shape
    xf = np.ascontiguousarray(x.reshape(-1, D).astype(np.float32, copy=False))
    idx = np.asarray(np_inputs["expert_idx"]).reshape(-1)
    w_gate = np.asarray(np_inputs["w_gate"], dtype=np.float32)
    w_up = np.asarray(np_inputs["w_up"], dtype=np.float32)
    w_down = np.asarray(np_inputs["w_down"], dtype=np.float32)

    order = np.argsort(idx, kind="stable")
    counts = np.bincount(idx, minlength=E).astype(np.int64)
    starts = np.zeros(E + 1, dtype=np.int64)
    np.cumsum(counts, out=starts[1:])
    cap = max(128, int(math.ceil(max(counts.max(), 1) / 128)) * 128)

    tok_of = [order[starts[e]:starts[e + 1]] for e in range(E)]
    x_of = [_prep_x(xf[tok_of[e]], cap) for e in range(E)]

    in_maps = []
    for pair in range(4):
        ea, eb = 2 * pair, 2 * pair + 1
        for hhalf in range(2):
            sl = slice(hhalf * F2, (hhalf + 1) * F2)
            in_maps.append({
                "xa": x_of[ea],
                "xb": x_of[eb],
                "wga": _prep_w_gate_like(w_gate[ea][:, sl]),
                "wua": _prep_w_gate_like(w_up[ea][:, sl]),
                "wda": _prep_w_down(w_down[ea][sl, :]),
                "wgb": _prep_w_gate_like(w_gate[eb][:, sl]),
                "wub": _prep_w_gate_like(w_up[eb][:, sl]),
                "wdb": _prep_w_down(w_down[eb][sl, :]),
            })
    meta = (tok_of, counts, (B, S), x.dtype)
    return in_maps, cap, meta


def unshard(results, cap, meta):
    tok_of, counts, (B, S), out_dtype = meta
    out = np.zeros((B * S, D), dtype=np.float32)
    for pair in range(4):
        ea, eb = 2 * pair, 2 * pair + 1
        r0 = results[2 * pair]
        r1 = results[2 * pair + 1]
        for e, key in ((ea, "ya"), (eb, "yb")):
            yt = (np.asarray(r0[key]) + np.asarray(r1[key])).reshape(D, cap)
            out[tok_of[e]] = yt[:, :counts[e]].T
    return out.reshape(B, S, D).astype(out_dtype, copy=False)


def kernel(x, expert_idx, w_gate, w_up, w_down):
    np_inputs = {"x": x, "expert_idx": expert_idx, "w_gate": w_gate,
                 "w_up": w_up, "w_down": w_down}
    in_maps, cap, meta = prepare_in_maps(np_inputs)
    nc = get_program(cap)
    res = run_bass_kernel_spmd(nc, in_maps, list(range(8)))
    return unshard(res.results, cap, meta)


# revision 7
# speedup vs baseline: 4.3093x; 1.4710x over previous
"""MoE SwiGLU experts kernel for Trainium2, 8 NeuronCores.

Strategy: expert-pair parallel with F-split.
  - Tokens are sorted by expert on the host (argsort of expert_idx).
  - Cores 2i and 2i+1 jointly own experts (2i, 2i+1): both cores get the
    same token set (experts 2i & 2i+1, padded to `cap` tokens each), but
    core 2i uses columns [0:2048] of w_gate/w_up (rows of w_down) and
    core 2i+1 uses columns [2048:4096].  Each core runs the full SwiGLU
    over its F-half; the host sums the two partial down-projections.
  - On-chip layout is feature-major (partitions = feature dim, free dim =
    tokens), so w_gate/w_up/w_down natural layouts serve directly as the
    stationary lhsT operand: out = lhsT.T @ rhs.  Zero on-chip transposes.
"""

import math

import numpy as np
import orjson

import concourse.bass as bass
import concourse.mybir as mybir
import concourse.tile as tile
from concourse import bass2jax
from concourse.bass_utils import run_bass_kernel_spmd

FP32 = mybir.dt.float32
FP32R = mybir.dt.float32r

D = 1024        # model dim
F = 4096        # ffn dim
F2 = F // 2     # per-core F half
E = 8           # experts
KD = D // 128   # 8  k-tiles over D
KF = F2 // 128  # 16 f-tiles over F half


# ---------------------------------------------------------------------------
# BIR legalizer: this container's walrus accepts at most ONE sync-wait per
# instruction.  Tile emits instructions with several waits; hoist the excess
# onto preceding EventSemaphore carrier instructions on the same engine
# (engines execute their stream in order, so waiting earlier is equivalent).
# ---------------------------------------------------------------------------

def _legalize_bir_waits(bir_bytes: bytes) -> bytes:
    bir = orjson.loads(bir_bytes)
    n_fix = 0
    for f in bir.get("functions", []):
        for b in f.get("blocks", []):
            out = []
            for inst in b.get("instructions", []):
                si = inst.get("sync_info")
                waits = (si or {}).get("on_wait") or []
                if len(waits) > 1:
                    keep = waits[-1:]
                    excess = waits[:-1]
                    for w in excess:
                        n_fix += 1
                        out.append({
                            "debug": inst.get("debug", 0),
                            "engine": inst["engine"],
                            "ins": [], "outs": [],
                            "name": f"wfix-{n_fix}-{inst['name']}",
                            "opcode": "EventSemaphore",
                            "sync_info": {"on_update": [], "on_wait": [w]},
                        })
                    si["on_wait"] = keep
                out.append(inst)
            b["instructions"] = out
    return orjson.dumps(bir)


_orig_decompress = bass2jax._decompress_ant_bir


def _patched_decompress(v):
    return _legalize_bir_waits(_orig_decompress(v))


bass2jax._decompress_ant_bir = _patched_decompress


# ---------------------------------------------------------------------------
# Device program (SPMD, identical across the 8 cores; per-core inputs differ)
# ---------------------------------------------------------------------------

def _chunks(cap):
    """Split the token free-dim into matmul chunks: <=512 wide (one fp32 PSUM
    bank), multiples of 128, as even as possible.  float32r matmuls run at
    full rate only when the moving free-dim is >=256, so even splits beat a
    512/512/128 tail split."""
    n = max(1, math.ceil(cap / 512))
    base = (cap // n) // 16 * 16
    rem = (cap - n * base) // 16
    widths = [base + 16] * rem + [base] * (n - rem)
    out = []
    c0 = 0
    for w in widths:
        out.append((c0, w))
        c0 += w
    return out


def build_program(cap: int, reps: int = 1) -> bass.Bass:
    nc = bass.Bass()

    # unit a = even expert of the pair, unit b = odd expert.
    xa = nc.declare_dram_parameter("xa", [KD, 128, cap], FP32R, isOutput=False)
    xb = nc.declare_dram_parameter("xb", [KD, 128, cap], FP32R, isOutput=False)
    wga = nc.declare_dram_parameter("wga", [KF, 128, KD, 128], FP32R, isOutput=False)
    wua = nc.declare_dram_parameter("wua", [KF, 128, KD, 128], FP32R, isOutput=False)
    wda = nc.declare_dram_parameter("wda", [KD, 128, KF, 128], FP32R, isOutput=False)
    wgb = nc.declare_dram_parameter("wgb", [KF, 128, KD, 128], FP32R, isOutput=False)
    wub = nc.declare_dram_parameter("wub", [KF, 128, KD, 128], FP32R, isOutput=False)
    wdb = nc.declare_dram_parameter("wdb", [KD, 128, KF, 128], FP32R, isOutput=False)
    ya = nc.declare_dram_parameter("ya", [KD, 128, cap], FP32, isOutput=True)
    yb = nc.declare_dram_parameter("yb", [KD, 128, cap], FP32, isOutput=True)

    units = [(xa, wga, wua, wda, ya), (xb, wgb, wub, wdb, yb)] * reps
    chunks = _chunks(cap)

    with tile.TileContext(nc) as tc:
        with (
            tc.tile_pool(name="xp", bufs=1) as xp,
            tc.tile_pool(name="hp", bufs=1) as hp,
            tc.tile_pool(name="wp", bufs=3) as wp,
            tc.tile_pool(name="sp", bufs=2) as sp,
            tc.tile_pool(name="yp", bufs=2) as yp,
            tc.tile_pool(name="ps", bufs=2, space=bass.MemorySpace.PSUM) as ps,
        ):
            for x_d, wg_d, wu_d, wd_d, y_d in units:
                # token activations, feature-major: x_t[k] = X^T[d-tile k]
                x_t = []
                for k in range(KD):
                    xt = xp.tile([128, cap], FP32R, tag=f"x{k}")
                    nc.sync.dma_start(xt[:], x_d[k])
                    x_t.append(xt)

                # H^T for this unit: [f (128 part), f-tile (16), tokens]
                h = hp.tile([128, KF, cap], FP32R, tag="h")

                # ---- phase 1: G/U = X @ Wg, X @ Wu ; H = silu(G) * U ----
                for fj in range(KF):
                    wgt = wp.tile([128, KD, 128], FP32R, tag="wg")
                    nc.sync.dma_start(wgt[:], wg_d[fj])
                    wut = wp.tile([128, KD, 128], FP32R, tag="wu")
                    nc.sync.dma_start(wut[:], wu_d[fj])
                    for (c0, w) in chunks:
                        g_ps = ps.tile([128, w], FP32, tag="g")
                        for k in range(KD):
                            nc.tensor.matmul(
                                g_ps[:, :w],
                                wgt[:, k, :], x_t[k][:, c0:c0 + w],
                                start=(k == 0), stop=(k == KD - 1),
                            )
                        u_ps = ps.tile([128, w], FP32, tag="u")
                        for k in range(KD):
                            nc.tensor.matmul(
                                u_ps[:, :w],
                                wut[:, k, :], x_t[k][:, c0:c0 + w],
                                start=(k == 0), stop=(k == KD - 1),
                            )
                        sg = sp.tile([128, w], FP32, tag="sg")
                        nc.scalar.activation(
                            sg[:, :w], g_ps[:, :w],
                            mybir.ActivationFunctionType.Silu,
                        )
                        nc.vector.tensor_mul(
                            h[:, fj, c0:c0 + w], sg[:, :w], u_ps[:, :w]
                        )

                # ---- phase 2: Y^T partial = Wd^T @ H^T (accum over f) ----
                for d in range(KD):
                    wdt = wp.tile([128, KF, 128], FP32R, tag="wd")
                    nc.sync.dma_start(wdt[:], wd_d[d])
                    y_sb = yp.tile([128, cap], FP32, tag="y")
                    for (c0, w) in chunks:
                        y_ps = ps.tile([128, w], FP32, tag="yp")
                        for fj in range(KF):
                            nc.tensor.matmul(
                                y_ps[:, :w],
                                wdt[:, fj, :], h[:, fj, c0:c0 + w],
                                start=(fj == 0), stop=(fj == KF - 1),
                            )
                        nc.vector.tensor_copy(y_sb[:, c0:c0 + w], y_ps[:, :w])
                    nc.sync.dma_start(y_d[d], y_sb[:])

    return nc


# ---------------------------------------------------------------------------
# Host-side sharding / unsharding
# ---------------------------------------------------------------------------

def _prep_w_gate_like(w_slice):
    """[D, F2] -> [KF, 128, KD, 128] so each f-tile is one contiguous DMA."""
    arr = w_slice.reshape(KD, 128, KF, 128)       # [k, p, fj, f]
    return np.ascontiguousarray(arr.transpose(2, 1, 0, 3))


def _prep_w_down(w_slice):
    """[F2, D] -> [KD, 128, KF, 128] so each d-tile is one contiguous DMA."""
    arr = w_slice.reshape(KF, 128, KD, 128)       # [fj, p, d, dc]
    return np.ascontiguousarray(arr.transpose(2, 1, 0, 3))


def _prep_x(x_tok, cap):
    """[count, D] tokens -> [KD, 128, cap] feature-major padded."""
    count = x_tok.shape[0]
    xt = np.zeros((D, cap), dtype=np.float32)
    xt[:, :count] = x_tok.T
    return np.ascontiguousarray(xt.reshape(KD, 128, cap))


_prog_cache = {}


def get_program(cap: int) -> bass.Bass:
    if cap not in _prog_cache:
        _prog_cache[cap] = build_program(cap)
    return _prog_cache[cap]


def prepare_in_maps(np_inputs):
    """Host-side sharding.  Returns (in_maps, cap, meta) where meta carries
    what's needed to unshard."""
    x = np.asarray(np_inputs["x"])
    B, S, _ = x.shape
    xf = np.ascontiguousarray(x.reshape(-1, D).astype(np.float32, copy=False))
    idx = np.asarray(np_inputs["expert_idx"]).reshape(-1)
    w_gate = np.asarray(np_inputs["w_gate"], dtype=np.float32)
    w_up = np.asarray(np_inputs["w_up"], dtype=np.float32)
    w_down = np.asarray(np_inputs["w_down"], dtype=np.float32)

    order = np.argsort(idx, kind="stable")
    counts = np.bincount(idx, minlength=E).astype(np.int64)
    starts = np.zeros(E + 1, dtype=np.int64)
    np.cumsum(counts, out=starts[1:])
    cap = max(128, int(math.ceil(max(counts.max(), 1) / 16)) * 16)

    tok_of = [order[starts[e]:starts[e + 1]] for e in range(E)]
    x_of = [_prep_x(xf[tok_of[e]], cap) for e in range(E)]

    in_maps = []
    for pair in range(4):
        ea, eb = 2 * pair, 2 * pair + 1
        for hhalf in range(2):
            sl = slice(hhalf * F2, (hhalf + 1) * F2)
            in_maps.append({
                "xa": x_of[ea],
                "xb": x_of[eb],
                "wga": _prep_w_gate_like(w_gate[ea][:, sl]),
                "wua": _prep_w_gate_like(w_up[ea][:, sl]),
                "wda": _prep_w_down(w_down[ea][sl, :]),
                "wgb": _prep_w_gate_like(w_gate[eb][:, sl]),
                "wub": _prep_w_gate_like(w_up[eb][:, sl]),
                "wdb": _prep_w_down(w_down[eb][sl, :]),
            })
    meta = (tok_of, counts, (B, S), x.dtype)
    return in_maps, cap, meta


def unshard(results, cap, meta):
    tok_of, counts, (B, S), out_dtype = meta
    out = np.zeros((B * S, D), dtype=np.float32)
    for pair in range(4):
        ea, eb = 2 * pair, 2 * pair + 1
        r0 = results[2 * pair]
        r1 = results[2 * pair + 1]
        for e, key in ((ea, "ya"), (eb, "yb")):
            yt = (np.asarray(r0[key]) + np.asarray(r1[key])).reshape(D, cap)
            out[tok_of[e]] = yt[:, :counts[e]].T
    return out.reshape(B, S, D).astype(out_dtype, copy=False)


def kernel(x, expert_idx, w_gate, w_up, w_down):
    np_inputs = {"x": x, "expert_idx": expert_idx, "w_gate": w_gate,
                 "w_up": w_up, "w_down": w_down}
    in_maps, cap, meta = prepare_in_maps(np_inputs)
    nc = get_program(cap)
    res = run_bass_kernel_spmd(nc, in_maps, list(range(8)))
    return unshard(res.results, cap, meta)
